# revision 17
# baseline (speedup 1.0000x reference)
# Trainium2 Bass kernel for nn_Net_38233798869763 (Mamba-ish net, L=1).
#
# Math (L=1 collapses the reference):
#   - causal depthwise conv over L=1 reduces to xc = xs0*conv_w[:,3] + conv_b
#   - the SSM scan reduces to y_ssm = delta * xs * (Bm . Cm)   (dA hits h0=0)
#   so each layer is:
#     rs   = rsqrt(mean(x^2) + eps)
#     xn   = x * rs                                  (norm_w folded into W_in)
#     xs   = silu(xn @ W_xs.T + conv_b); sz = silu(xn @ W_z.T)
#     dbl  = xs @ x_proj_w.T; dlo, Bm, Cm = split(dbl)
#     s    = sum(Bm*Cm) = ((Bm+Cm)^2 - (Bm-Cm)^2)/4  (x_proj folded to p/m cols
#                                                     so the dot is all PE ops)
#     delta= softplus(dlo @ dt_w.T + dt_b)           (= Ln(Exp(u)+1) on ACT)
#     x   += ((delta*s + D_ssm) * xs * sz) @ out_w.T
#
# Precision plan (validated by host sim, rel_l2 ~1.1e-2 < 2e-2):
#   - trunk matmuls (in_proj / x_proj / out_proj) in fp8e4 with DoubleRow
#     (2 fp8 weights per PE cell -> ~1.4x matmul throughput), weights
#     host-scaled by power-of-2 per matrix, descale folded into the psum
#     evacuation activations; activations quantized to fp8 at evac time.
#   - base-signal matmuls (proj MLP, dt, dense MLP) stay bf16: fp8 there
#     costs ~4% output error (no residual damping), bf16 is cheap (44us).
# Batch sharded across 8 cores (512 rows/core), feature-on-partitions.
import numpy as np
import ml_dtypes

B, IN, D, OUT = 4096, 512, 1024, 256
NL, DI, N, DCONV, DTR = 4, 2048, 16, 4, 64
NCORES = 8
BL = B // NCORES          # 512 batch rows per core
KD = D // 128             # 8   k-tiles over D
KIN = IN // 128           # 4   k-tiles over IN
KDI = DI // 128           # 16  k-tiles over DI
JI = 2 * DI // 128        # 32  j-tiles of in_proj output
GJ = 8                    # j-tiles per psum group
NG = JI // GJ             # 4   groups (2 xs + 2 z)

XPW = DTR + 3 * N        # x_proj out cols: dlo|p|pad|m (32-aligned starts)
S_XS = 2048.0             # fp8 scale: in_proj xs half (conv tap folded in)
S_Z = 256.0               # fp8 scale: in_proj z half
S_XP = 256.0              # fp8 scale: x_proj
S_OUT = 2048.0            # fp8 scale: out_proj
S_X = 16.0                # fp8 scale of normalized activations (via rs)

_cache = {}


def _q8(a, s):
    f8 = ml_dtypes.float8_e4m3
    return np.clip(np.asarray(a, np.float32) * s, -240.0, 240.0).astype(f8)


def _host_pack(inputs):
    bfl = ml_dtypes.bfloat16
    f32 = np.float32

    def t(a):
        return np.ascontiguousarray(a)

    p = {}
    # proj MLP (bf16)
    p["w_p1"] = t(inputs["pw1"].T.reshape(KIN, 128, D // 2).transpose(1, 0, 2).astype(bfl))
    p["b_p1"] = t(inputs["pb1"].reshape(D // 2 // 128, 128).T.astype(f32))
    p["w_p2"] = t(inputs["pw2"].T.reshape(KIN, 128, D).transpose(1, 0, 2).astype(bfl))
    p["b_p2"] = t(inputs["pb2"].reshape(KD, 128).T.astype(f32))
    # dense MLP (bf16)
    dw1T = inputs["dw1"].T            # [D, 2D]
    p["w_d1"] = t(np.stack([
        dw1T[:, g * 1024:(g + 1) * 1024].reshape(KD, 128, 1024).transpose(1, 0, 2)
        for g in range(2)
    ]).astype(bfl))                   # [2, 128, 8, 1024]
    p["b_d1"] = t(inputs["db1"].reshape(16, 128).T.astype(f32))
    p["w_d2"] = t(inputs["dw2"].T.reshape(16, 128, OUT).transpose(1, 0, 2).astype(bfl))
    p["b_d2"] = t(inputs["db2"].reshape(2, 128).T.astype(f32))
    # per-layer mamba params
    for l in range(NL):
        W_in = inputs["in_proj_w"][l] * inputs["norm_w"][l][None, :]
        W_in = W_in.copy()
        W_in[:DI] *= inputs["conv_w"][l][:, DCONV - 1][:, None]   # fold last conv tap
        W_in[:DI] *= S_XS
        W_in[DI:] *= S_Z
        WT = np.clip(W_in, -240.0, 240.0).T                       # [D, 2*DI] scaled
        p[f"w_in{l}"] = t(np.stack([
            WT[:, g * 1024:(g + 1) * 1024].reshape(KD, 128, 1024).transpose(1, 0, 2)
            for g in range(NG)
        ]).astype(ml_dtypes.float8_e4m3))                         # [4, 128, 8, 1024] fp8
        # x_proj folded: [dlo(64) | Bm+Cm(16) | Bm-Cm(16)]
        Wxp = inputs["x_proj_w"][l]
        Wxp_pm = np.concatenate([Wxp[:DTR],
                                 Wxp[DTR:DTR + N] + Wxp[DTR + N:],
                                 np.zeros((N, DI), np.float32),
                                 Wxp[DTR:DTR + N] - Wxp[DTR + N:]], axis=0)
        p[f"w_xp{l}"] = t(Wxp_pm.T.astype(bfl).reshape(KDI, 128, XPW)
                          .transpose(1, 0, 2))                    # [128, 16, 112] bf16
        p[f"w_dt{l}"] = t(inputs["dt_w"][l].T.reshape(DTR, KDI, 128).astype(bfl))
        p[f"w_out{l}"] = t(_q8(inputs["out_w"][l].T, S_OUT).reshape(KDI, 128, D)
                           .transpose(1, 0, 2))                   # [128, 16, 1024] fp8
        p[f"b_cv{l}"] = t(inputs["conv_b"][l].reshape(KDI, 128).T.astype(f32))
        p[f"b_dt{l}"] = t(inputs["dt_b"][l].reshape(KDI, 128).T.astype(f32))
        p[f"d_ssm{l}"] = t(inputs["D_ssm"][l].reshape(KDI, 128).T.astype(f32))
    # input, transposed + per-core sliced: x^T [IN, B] -> [core][128, KIN, BL]
    xT = inputs["x"].T.astype(bfl)
    xc = []
    for c in range(NCORES):
        s = xT[:, c * BL:(c + 1) * BL].reshape(KIN, 128, BL).transpose(1, 0, 2)
        xc.append(t(s))
    return p, xc


def _patch_act_tables():
    """Steer the ACT table-set chooser: Exp+Ln co-reside (softplus + rms
    stats both live in natural_log_exp_and_others), Tanh+Silu co-reside in
    silu_and_others. Dict ORDER and SIZE must stay identical to
    act_info.json (set ids are positional); only MEMBERSHIP is edited."""
    import concourse.mybir as mybir
    import concourse.bacc as bacc_mod
    if getattr(bacc_mod, "_act_tables_patched", False):
        return
    orig = bacc_mod.get_activation_tables
    AF = mybir.ActivationFunctionType

    def steered(module_arch):
        tabs = orig(module_arch)
        keep = "natural_log_exp_and_others"
        for name, fns in tabs.items():
            if name != keep:
                fns.discard(AF.Exp)
                fns.discard(AF.Ln)
            if name != "silu_and_others":
                fns.discard(AF.Tanh)
        return tabs

    bacc_mod.get_activation_tables = steered
    bacc_mod._act_tables_patched = True


def _build():
    import concourse.tile as tile
    import concourse.mybir as mybir
    from concourse import bacc

    _patch_act_tables()

    dt = mybir.dt
    AF = mybir.ActivationFunctionType
    ALU = mybir.AluOpType
    DR = mybir.MatmulPerfMode.DoubleRow

    nc = bacc.Bacc("TRN2", target_bir_lowering=False, debug=False,
                   num_devices=NCORES)

    def din(name, shape, dtp):
        return nc.dram_tensor(name, shape, dtp, kind="ExternalInput").ap()

    x_in = din("x_in", [128, KIN, BL], dt.bfloat16)
    w_p1 = din("w_p1", [128, KIN, D // 2], dt.bfloat16)
    b_p1 = din("b_p1", [128, KIN], dt.float32)
    w_p2 = din("w_p2", [128, KIN, D], dt.bfloat16)
    b_p2 = din("b_p2", [128, KD], dt.float32)
    w_d1 = din("w_d1", [2, 128, KD, 1024], dt.bfloat16)
    b_d1 = din("b_d1", [128, 16], dt.float32)
    w_d2 = din("w_d2", [128, 16, OUT], dt.bfloat16)
    b_d2 = din("b_d2", [128, 2], dt.float32)
    w_in = [din(f"w_in{l}", [NG, 128, KD, 1024], dt.float8e4) for l in range(NL)]
    w_xp = [din(f"w_xp{l}", [128, KDI, XPW], dt.bfloat16) for l in range(NL)]
    w_dt = [din(f"w_dt{l}", [DTR, KDI, 128], dt.bfloat16) for l in range(NL)]
    w_out = [din(f"w_out{l}", [128, KDI, 1024], dt.float8e4) for l in range(NL)]
    b_cv = [din(f"b_cv{l}", [128, KDI], dt.float32) for l in range(NL)]
    b_dt = [din(f"b_dt{l}", [128, KDI], dt.float32) for l in range(NL)]
    d_ssm = [din(f"d_ssm{l}", [128, KDI], dt.float32) for l in range(NL)]
    out_d = nc.dram_tensor("out", [2, 128, BL], dt.bfloat16, kind="ExternalOutput").ap()

    with tile.TileContext(nc) as tc:
        with (
            tc.tile_pool(name="singles", bufs=1) as sing,
            tc.tile_pool(name="wg", bufs=2) as wgp,
            tc.tile_pool(name="wgd", bufs=2) as wgdp,
            tc.tile_pool(name="wout", bufs=2) as wwp,
            tc.tile_pool(name="tmp", bufs=1) as tmpp,
            tc.tile_pool(name="ps", bufs=1, space="PSUM") as ps,
        ):
            # ---- constants ----
            eps_t = sing.tile([1, 1], dt.float32)
            nc.vector.memset(eps_t[:], 1e-5)
            ln16_t = sing.tile([1, 1], dt.float32)
            nc.vector.memset(ln16_t[:], float(np.log(S_X)))
            ones8 = sing.tile([128, 1], dt.float8e4)
            nc.vector.memset(ones8[:], 1.0)
            ones1_bf = sing.tile([1, 128], dt.bfloat16)
            nc.vector.memset(ones1_bf[:], 1.0)
            pm_w = sing.tile([XPW, 128], dt.bfloat16)    # +-1/4 rows for s dot
            nc.vector.memset(pm_w[64:XPW, :], 0.0)
            nc.vector.memset(pm_w[64:80, :], 0.25)
            nc.vector.memset(pm_w[96:112, :], -0.25)
            junk = sing.tile([1, 8], dt.float32)
            nc.vector.memset(junk[:], 0.0)

            def act_dummy(func, dep=None):
                # tiny op to pull the ACT table load into engine slack;
                # dep (an AP) orders it after the producer so the tile
                # scheduler cannot hoist it ahead of the previous table era
                src_ap = junk[:] if dep is None else dep
                nc.scalar.activation(junk[:], src_ap, func)

            act_dummy(AF.Tanh)     # pre-load the silu/tanh table set at t=0

            # ---- resident small weights / inputs ----
            # proj weights + input share the dense-weight pool buffers
            # (dead after proj phase; dense DMAs recycle them)
            xw1_sb = wgdp.tile([128, KIN, BL + D // 2], dt.bfloat16, tag="wgd",
                               name="xw1")
            x_sb = xw1_sb[:, :, :BL]
            wp1_sb = xw1_sb[:, :, BL:]
            for k in range(KIN):
                nc.sync.dma_start(x_sb[:, k, :], x_in[:, k])
                nc.sync.dma_start(wp1_sb[:, k, :], w_p1[:, k])
            wp2_sb = wgdp.tile([128, KIN, D], dt.bfloat16, tag="wgd", name="wp2")
            nc.sync.dma_start(wp2_sb[:], w_p2)
            bp1_sb = sing.tile([128, KIN], dt.float32)
            nc.sync.dma_start(bp1_sb[:], b_p1)
            bp2_sb = sing.tile([128, KD], dt.float32)
            nc.sync.dma_start(bp2_sb[:], b_p2)
            bd1_sb = sing.tile([128, 16], dt.float32)
            nc.sync.dma_start(bd1_sb[:], b_d1)
            wd2_sb = sing.tile([128, 16, OUT], dt.bfloat16)
            nc.sync.dma_start(wd2_sb[:], w_d2)
            bd2_sb = sing.tile([128, 2], dt.float32)
            nc.sync.dma_start(bd2_sb[:], b_d2)

            # ---- persistent activations ----
            xT = sing.tile([128, KD, BL], dt.float32)       # residual stream x^T
            x_bf = sing.tile([128, KD, BL], dt.bfloat16)    # h1 scratch + dense rhs
            scr8 = sing.tile([128, KD, BL], dt.float8e4)    # squares, then xn8
            xs_bf = sing.tile([128, KDI, BL], dt.bfloat16)
            sz_bf = sing.tile([128, KDI, BL], dt.bfloat16)
            xssz = sing.tile([128, KDI, BL], dt.bfloat16)
            delta = sing.tile([128, KDI, BL], dt.bfloat16)  # also dense g1 scratch
            yin8 = sing.tile([128, KDI, BL], dt.float8e4)
            dlo_bf = sing.tile([DTR, BL], dt.bfloat16)
            sqpm = sing.tile([XPW, BL], dt.bfloat16)
            s_sb = sing.tile([128, BL], dt.bfloat16)
            rs_bf = sing.tile([1, BL], dt.bfloat16)
            lnms_t = sing.tile([1, BL], dt.float32)
            out_sb = sing.tile([128, 2, BL], dt.bfloat16)

            _psn = [0]

            def mm_ps(tag="mm", bufs=6, shape=(128, BL)):
                _psn[0] += 1
                return ps.tile(list(shape), dt.float32, tag=tag, bufs=bufs,
                               name=f"ps_{tag}_{_psn[0]}")

            # ======== proj MLP: x -> h1 -> x_T (+ squares for L0 rms) ====
            with nc.named_scope("proj_mlp"):
                for j in range(KIN):        # h1 j-tiles (D/2 = 512 -> 4)
                    pt = mm_ps()
                    for k in range(KIN):
                        nc.tensor.matmul(pt[:], wp1_sb[:, k, j * 128:(j + 1) * 128],
                                         x_sb[:, k, :],
                                         start=(k == 0), stop=(k == KIN - 1))
                    nc.scalar.activation(x_bf[:, j, :], pt[:], AF.Tanh,
                                         bias=bp1_sb[:, j:j + 1])
                h1_bf = x_bf                # h1 lives in x_bf[:, 0:4, :]
                for j in range(KD):         # h j-tiles (D = 1024 -> 8)
                    pt = mm_ps()
                    for k in range(KIN):
                        nc.tensor.matmul(pt[:], wp2_sb[:, k, j * 128:(j + 1) * 128],
                                         h1_bf[:, k, :],
                                         start=(k == 0), stop=(k == KIN - 1))
                    nc.scalar.activation(xT[:, j, :], pt[:], AF.Identity,
                                         bias=bp2_sb[:, j:j + 1])
                    nc.vector.scalar_tensor_tensor(
                        scr8[:, j, :], xT[:, j, :], 4.0, xT[:, j, :],
                        ALU.mult, ALU.mult)

            # ======== mamba layers ========
            for l in range(NL):
                with nc.named_scope(f"L{l}_pre"):
                    wxp = tmpp.tile([128, KDI, XPW], dt.bfloat16, tag="wxp")
                    nc.sync.dma_start(wxp[:], w_xp[l])
                    wdt = tmpp.tile([DTR, KDI, 128], dt.bfloat16, tag="wdt")
                    nc.sync.dma_start(wdt[:], w_dt[l])
                    bcv = tmpp.tile([128, KDI], dt.float32, tag="bcv")
                    nc.sync.dma_start(bcv[:], b_cv[l])
                    bdt = tmpp.tile([128, KDI], dt.float32, tag="bdt")
                    nc.sync.dma_start(bdt[:], b_dt[l])
                    dsm = tmpp.tile([128, KDI], dt.float32, tag="dsm")
                    nc.sync.dma_start(dsm[:], d_ssm[l])
                    wout = wwp.tile([128, KDI, 1024], dt.float8e4, tag="wout")
                    nc.sync.dma_start(wout[:], w_out[l])

                    # rms stats on fp8 squares; xn8 = xT * (rs*16) in fp8
                    pssq = mm_ps(tag="small", bufs=2, shape=(1, BL))
                    for k in range(KD):
                        nc.tensor.matmul(pssq[:], ones8[:], scr8[:, k, :],
                                         start=(k == 0), stop=(k == KD - 1))
                    nc.scalar.activation(lnms_t[:], pssq[:], AF.Ln,
                                         bias=eps_t[:], scale=1.0 / (4 * D))
                    nc.scalar.activation(rs_bf[:], lnms_t[:], AF.Exp,
                                         bias=ln16_t[:], scale=-0.5)
                    act_dummy(AF.Silu, rs_bf[:, 0:8])
                    ps_rs = mm_ps(tag="small", bufs=2)
                    nc.tensor.matmul(ps_rs[:], ones1_bf[:], rs_bf[:],
                                     start=True, stop=True)
                    xn8 = scr8
                    for k in range(KD):
                        nc.vector.tensor_mul(xn8[:, k, :], xT[:, k, :], ps_rs[:])

                # --- in_proj: fp8 DoubleRow, descale folded into silu evac ---
                def xproj_chain():
                    # x_proj (bf16) -> dlo | p | m ; s = (p^2 - m^2)/4.
                    # Identity/Square run in any ACT table era, so this
                    # overlaps the z-half silu block without table thrash.
                    with nc.named_scope(f"L{l}_xproj"):
                        pdb = mm_ps(tag="small", bufs=2, shape=(XPW, BL))
                        for k in range(KDI):
                            nc.tensor.matmul(pdb[:], wxp[:, k, :], xs_bf[:, k, :],
                                             start=(k == 0), stop=(k == KDI - 1))
                        nc.scalar.activation(dlo_bf[:], pdb[:DTR, :], AF.Identity)
                        nc.scalar.activation(sqpm[64:XPW, :], pdb[64:XPW, :],
                                             AF.Square)
                        ps_s = mm_ps(tag="small", bufs=2)
                        nc.tensor.matmul(ps_s[:], pm_w[64:XPW, :], sqpm[64:XPW, :],
                                         start=True, stop=True)
                        nc.scalar.activation(s_sb[:], ps_s[:], AF.Identity)

                with nc.named_scope(f"L{l}_inproj"):
                    for g in range(NG):
                        if g == 2:
                            xproj_chain()
                        wg = wgp.tile([128, KD, 1024], dt.float8e4, tag="wg")
                        nc.sync.dma_start(wg[:], w_in[l][g])
                        for jj in range(GJ):
                            pt = mm_ps(tag="mm", bufs=6)
                            for kk in range(0, KD, 2):
                                nc.tensor.matmul(
                                    pt[:], wg[:, kk:kk + 2, jj * 128:(jj + 1) * 128],
                                    xn8[:, kk:kk + 2, :],
                                    start=(kk == 0), stop=(kk == KD - 2),
                                    perf_mode=DR)
                            j = g * GJ + jj
                            if j < KDI:
                                nc.scalar.activation(xs_bf[:, j, :], pt[:], AF.Silu,
                                                     bias=bcv[:, j:j + 1],
                                                     scale=1.0 / (S_XS * S_X))
                            else:
                                nc.scalar.activation(sz_bf[:, j - KDI, :], pt[:],
                                                     AF.Silu,
                                                     scale=1.0 / (S_Z * S_X))

                with nc.named_scope(f"L{l}_xssz"):
                    # split between DVE (fast all-bf16 2x path) and GpSimd so
                    # neither lane gates the downstream STT chain
                    for k in range(KDI):
                        eng = nc.vector if k % 2 == 0 else nc.gpsimd
                        eng.tensor_mul(xssz[:, k, :], xs_bf[:, k, :],
                                       sz_bf[:, k, :])

                if l == 1:
                    # prefetch dense-MLP weights early; pool buffers are the
                    # recycled proj-weight buffers, free since the proj phase
                    dense_wg = []
                    for g in range(2):
                        wgd = wgdp.tile([128, KD, 1024], dt.bfloat16, tag="wgd",
                                        name=f"dense_wg{g}")
                        nc.sync.dma_start(wgd[:], w_d1[g])
                        dense_wg.append(wgd)

                # --- dt (bf16): softplus = Ln(Exp(u+b)+1), Ln paired ---
                with nc.named_scope(f"L{l}_dt"):
                    act_dummy(AF.Exp, sz_bf[0:1, KDI - 1, 0:8])
                    for j in range(KDI):
                        pt = mm_ps()
                        nc.tensor.matmul(pt[:], wdt[:, j, :], dlo_bf[:],
                                         start=True, stop=True)
                        nc.scalar.activation(delta[:, j, :], pt[:], AF.Exp,
                                             bias=bdt[:, j:j + 1])
                        if j % 2 == 1:
                            nc.scalar.activation(delta[:, j - 1:j + 1, :],
                                                 delta[:, j - 1:j + 1, :],
                                                 AF.Ln, bias=1.0)
                            nc.gpsimd.tensor_mul(delta[:, j - 1, :],
                                                   delta[:, j - 1, :], s_sb[:])
                            nc.vector.tensor_mul(delta[:, j, :],
                                                 delta[:, j, :], s_sb[:])
                            for jj in (j - 1, j):
                                nc.vector.scalar_tensor_tensor(
                                    delta[:, jj, :], delta[:, jj, :],
                                    dsm[:, jj:jj + 1], xssz[:, jj, :],
                                    ALU.add, ALU.mult)
                            nc.vector.tensor_copy(yin8[:, j - 1:j + 1, :],
                                                  delta[:, j - 1:j + 1, :])

                # --- y @ out_w (fp8 DR) + residual; squares for next rms ---
                with nc.named_scope(f"L{l}_y_out"):
                    NP1 = 6
                    pouts = [mm_ps() for _ in range(NP1)]
                    for kk in range(0, KDI, 2):
                        for j in range(NP1):
                            nc.tensor.matmul(pouts[j][:],
                                             wout[:, kk:kk + 2, j * 128:(j + 1) * 128],
                                             yin8[:, kk:kk + 2, :],
                                             start=(kk == 0), stop=(kk == KDI - 2),
                                             perf_mode=DR)

                    def evac_y(j, pt):
                        if l < NL - 1:
                            nc.vector.scalar_tensor_tensor(
                                xT[:, j, :], pt[:], 1.0 / S_OUT, xT[:, j, :],
                                ALU.mult, ALU.add)
                            nc.scalar.activation(scr8[:, j, :], xT[:, j, :],
                                                 AF.Square, scale=2.0)
                        else:
                            nc.vector.scalar_tensor_tensor(
                                x_bf[:, j, :], pt[:], 1.0 / S_OUT, xT[:, j, :],
                                ALU.mult, ALU.add)

                    for j in range(NP1):
                        evac_y(j, pouts[j])
                    for j in range(NP1, KD):
                        pt = mm_ps()
                        for kk in range(0, KDI, 2):
                            nc.tensor.matmul(pt[:],
                                             wout[:, kk:kk + 2, j * 128:(j + 1) * 128],
                                             yin8[:, kk:kk + 2, :],
                                             start=(kk == 0), stop=(kk == KDI - 2),
                                             perf_mode=DR)
                        evac_y(j, pt)

            # ======== dense MLP (bf16): x -> g1 -> out ========
            with nc.named_scope("dense_mlp"):
                act_dummy(AF.Tanh, x_bf[0:1, KD - 1, 0:8])
                g1_bf = delta               # reuse [128, KDI, BL] bf16 scratch
                for g in range(2):
                    wgd = dense_wg[g]
                    for jj in range(GJ):
                        pt = mm_ps()
                        for k in range(KD):
                            nc.tensor.matmul(pt[:], wgd[:, k, jj * 128:(jj + 1) * 128],
                                             x_bf[:, k, :],
                                             start=(k == 0), stop=(k == KD - 1))
                        j = g * GJ + jj
                        nc.scalar.activation(g1_bf[:, j, :], pt[:], AF.Tanh,
                                             bias=bd1_sb[:, j:j + 1])
                for j in range(2):
                    pt = mm_ps()
                    for k in range(16):
                        nc.tensor.matmul(pt[:], wd2_sb[:, k, j * 128:(j + 1) * 128],
                                         g1_bf[:, k, :], start=(k == 0),
                                         stop=(k == 15))
                    nc.scalar.activation(out_sb[:, j, :], pt[:], AF.Tanh,
                                         bias=bd2_sb[:, j:j + 1])
                    nc.gpsimd.dma_start(out_d[j], out_sb[:, j, :])

    nc.compile()
    return nc


def _run(inputs, trace=False, trace_kwargs=None):
    if "nc" not in _cache:
        _cache["nc"] = _build()
    nc = _cache["nc"]
    p, xc = _host_pack(inputs)
    in_maps = []
    for c in range(NCORES):
        m = dict(p)
        m["x_in"] = xc[c]
        in_maps.append(m)

    from concourse.bass_utils import run_bass_kernel_spmd
    kw = {}
    if trace:
        kw.update(trace=True, trace_cores=[0], trace_kwargs=trace_kwargs or {})
    res = run_bass_kernel_spmd(nc, in_maps, core_ids=list(range(NCORES)), **kw)

    # assemble: per core out [2, 128, BL] -> out^T [256, BL] -> [BL, 256]
    full = np.empty((B, OUT), np.float32)
    for c in range(NCORES):
        o = res.results[c]["out"].reshape(OUT, BL).astype(np.float32)
        full[c * BL:(c + 1) * BL] = o.T
    return full.reshape(-1), res


def kernel(**inputs):
    out, _ = _run(inputs, trace=False)
    return out


# revision 18
# speedup vs baseline: 1.0605x; 1.0605x over previous
# Trainium2 Bass kernel for nn_Net_38233798869763 (Mamba-ish net, L=1).
#
# Math (L=1 collapses the reference):
#   - causal depthwise conv over L=1 reduces to xc = xs0*conv_w[:,3] + conv_b
#   - the SSM scan reduces to y_ssm = delta * xs * (Bm . Cm)   (dA hits h0=0)
#   so each layer is:
#     rs   = rsqrt(mean(x^2) + eps)
#     xn   = x * rs                                  (norm_w folded into W_in)
#     xs   = silu(xn @ W_xs.T + conv_b); sz = silu(xn @ W_z.T)
#     dbl  = xs @ x_proj_w.T; dlo, Bm, Cm = split(dbl)
#     s    = sum(Bm*Cm) = ((Bm+Cm)^2 - (Bm-Cm)^2)/4  (x_proj folded to p/m cols
#                                                     so the dot is all PE ops)
#     delta= softplus(dlo @ dt_w.T + dt_b)           (= Ln(Exp(u)+1) on ACT)
#     x   += ((delta*s + D_ssm) * xs * sz) @ out_w.T
#
# Precision plan (validated by host sim, rel_l2 ~1.1e-2 < 2e-2):
#   - trunk matmuls (in_proj / x_proj / out_proj) in fp8e4 with DoubleRow
#     (2 fp8 weights per PE cell -> ~1.4x matmul throughput), weights
#     host-scaled by power-of-2 per matrix, descale folded into the psum
#     evacuation activations; activations quantized to fp8 at evac time.
#   - base-signal matmuls (proj MLP, dt, dense MLP) stay bf16: fp8 there
#     costs ~4% output error (no residual damping), bf16 is cheap (44us).
# Batch sharded across 8 cores (512 rows/core), feature-on-partitions.
import numpy as np
import ml_dtypes

B, IN, D, OUT = 4096, 512, 1024, 256
NL, DI, N, DCONV, DTR = 4, 2048, 16, 4, 64
NCORES = 8
BL = B // NCORES          # 512 batch rows per core
KD = D // 128             # 8   k-tiles over D
KIN = IN // 128           # 4   k-tiles over IN
KDI = DI // 128           # 16  k-tiles over DI
JI = 2 * DI // 128        # 32  j-tiles of in_proj output
GJ = 8                    # j-tiles per psum group
NG = JI // GJ             # 4   groups (2 xs + 2 z)

XPW = DTR + 3 * N        # x_proj out cols: dlo|p|pad|m (32-aligned starts)
S_XS = 2048.0             # fp8 scale: in_proj xs half (conv tap folded in)
S_Z = 256.0               # fp8 scale: in_proj z half
S_XP = 256.0              # fp8 scale: x_proj
S_OUT = 2048.0            # fp8 scale: out_proj
S_X = 16.0                # fp8 scale of normalized activations (via rs)

_cache = {}


def _q8(a, s):
    f8 = ml_dtypes.float8_e4m3
    return np.clip(np.asarray(a, np.float32) * s, -240.0, 240.0).astype(f8)


def _host_pack(inputs):
    bfl = ml_dtypes.bfloat16
    f32 = np.float32

    def t(a):
        return np.ascontiguousarray(a)

    p = {}
    # proj MLP (bf16)
    p["w_p1"] = t(inputs["pw1"].T.reshape(KIN, 128, D // 2).transpose(1, 0, 2).astype(bfl))
    p["b_p1"] = t(inputs["pb1"].reshape(D // 2 // 128, 128).T.astype(f32))
    p["w_p2"] = t(inputs["pw2"].T.reshape(KIN, 128, D).transpose(1, 0, 2).astype(bfl))
    p["b_p2"] = t(inputs["pb2"].reshape(KD, 128).T.astype(f32))
    # dense MLP (bf16)
    dw1T = inputs["dw1"].T            # [D, 2D]
    p["w_d1"] = t(np.stack([
        dw1T[:, g * 1024:(g + 1) * 1024].reshape(KD, 128, 1024).transpose(1, 0, 2)
        for g in range(2)
    ]).astype(bfl))                   # [2, 128, 8, 1024]
    p["b_d1"] = t(inputs["db1"].reshape(16, 128).T.astype(f32))
    p["w_d2"] = t(inputs["dw2"].T.reshape(16, 128, OUT).transpose(1, 0, 2).astype(bfl))
    p["b_d2"] = t(inputs["db2"].reshape(2, 128).T.astype(f32))
    # per-layer mamba params
    for l in range(NL):
        W_in = inputs["in_proj_w"][l] * inputs["norm_w"][l][None, :]
        W_in = W_in.copy()
        W_in[:DI] *= inputs["conv_w"][l][:, DCONV - 1][:, None]   # fold last conv tap
        W_in[:DI] *= S_XS
        W_in[DI:] *= S_Z
        WT = np.clip(W_in, -240.0, 240.0).T                       # [D, 2*DI] scaled
        p[f"w_in{l}"] = t(np.stack([
            WT[:, g * 1024:(g + 1) * 1024].reshape(KD, 128, 1024).transpose(1, 0, 2)
            for g in range(NG)
        ]).astype(ml_dtypes.float8_e4m3))                         # [4, 128, 8, 1024] fp8
        # x_proj folded: [dlo(64) | Bm+Cm(16) | Bm-Cm(16)]
        Wxp = inputs["x_proj_w"][l]
        Wxp_pm = np.concatenate([Wxp[:DTR],
                                 Wxp[DTR:DTR + N] + Wxp[DTR + N:],
                                 np.zeros((N, DI), np.float32),
                                 Wxp[DTR:DTR + N] - Wxp[DTR + N:]], axis=0)
        p[f"w_xp{l}"] = t(Wxp_pm.T.astype(bfl).reshape(KDI, 128, XPW)
                          .transpose(1, 0, 2))                    # [128, 16, 112] bf16
        p[f"w_dt{l}"] = t(inputs["dt_w"][l].T.reshape(DTR, KDI, 128).astype(bfl))
        p[f"w_out{l}"] = t(_q8(inputs["out_w"][l].T, S_OUT).reshape(KDI, 128, D)
                           .transpose(1, 0, 2))                   # [128, 16, 1024] fp8
        p[f"b_cv{l}"] = t(inputs["conv_b"][l].reshape(KDI, 128).T.astype(f32))
        p[f"b_dt{l}"] = t(inputs["dt_b"][l].reshape(KDI, 128).T.astype(f32))
        p[f"d_ssm{l}"] = t(inputs["D_ssm"][l].reshape(KDI, 128).T.astype(f32))
    # input, transposed + per-core sliced: x^T [IN, B] -> [core][128, KIN, BL]
    xT = inputs["x"].T.astype(bfl)
    xc = []
    for c in range(NCORES):
        s = xT[:, c * BL:(c + 1) * BL].reshape(KIN, 128, BL).transpose(1, 0, 2)
        xc.append(t(s))
    return p, xc


def _patch_act_tables():
    """Steer the ACT table-set chooser: Exp+Ln co-reside (softplus + rms
    stats both live in natural_log_exp_and_others), Tanh+Silu co-reside in
    silu_and_others. Dict ORDER and SIZE must stay identical to
    act_info.json (set ids are positional); only MEMBERSHIP is edited."""
    import concourse.mybir as mybir
    import concourse.bacc as bacc_mod
    if getattr(bacc_mod, "_act_tables_patched", False):
        return
    orig = bacc_mod.get_activation_tables
    AF = mybir.ActivationFunctionType

    def steered(module_arch):
        tabs = orig(module_arch)
        keep = "natural_log_exp_and_others"
        for name, fns in tabs.items():
            if name != keep:
                fns.discard(AF.Exp)
                fns.discard(AF.Ln)
            if name != "silu_and_others":
                fns.discard(AF.Tanh)
        return tabs

    bacc_mod.get_activation_tables = steered
    bacc_mod._act_tables_patched = True


def _build():
    import concourse.tile as tile
    import concourse.mybir as mybir
    from concourse import bacc

    _patch_act_tables()

    dt = mybir.dt
    AF = mybir.ActivationFunctionType
    ALU = mybir.AluOpType
    DR = mybir.MatmulPerfMode.DoubleRow

    nc = bacc.Bacc("TRN2", target_bir_lowering=False, debug=False,
                   num_devices=NCORES)

    def din(name, shape, dtp):
        return nc.dram_tensor(name, shape, dtp, kind="ExternalInput").ap()

    x_in = din("x_in", [128, KIN, BL], dt.bfloat16)
    w_p1 = din("w_p1", [128, KIN, D // 2], dt.bfloat16)
    b_p1 = din("b_p1", [128, KIN], dt.float32)
    w_p2 = din("w_p2", [128, KIN, D], dt.bfloat16)
    b_p2 = din("b_p2", [128, KD], dt.float32)
    w_d1 = din("w_d1", [2, 128, KD, 1024], dt.bfloat16)
    b_d1 = din("b_d1", [128, 16], dt.float32)
    w_d2 = din("w_d2", [128, 16, OUT], dt.bfloat16)
    b_d2 = din("b_d2", [128, 2], dt.float32)
    w_in = [din(f"w_in{l}", [NG, 128, KD, 1024], dt.float8e4) for l in range(NL)]
    w_xp = [din(f"w_xp{l}", [128, KDI, XPW], dt.bfloat16) for l in range(NL)]
    w_dt = [din(f"w_dt{l}", [DTR, KDI, 128], dt.bfloat16) for l in range(NL)]
    w_out = [din(f"w_out{l}", [128, KDI, 1024], dt.float8e4) for l in range(NL)]
    b_cv = [din(f"b_cv{l}", [128, KDI], dt.float32) for l in range(NL)]
    b_dt = [din(f"b_dt{l}", [128, KDI], dt.float32) for l in range(NL)]
    d_ssm = [din(f"d_ssm{l}", [128, KDI], dt.float32) for l in range(NL)]
    out_d = nc.dram_tensor("out", [2, 128, BL], dt.bfloat16, kind="ExternalOutput").ap()

    with tile.TileContext(nc) as tc:
        with (
            tc.tile_pool(name="singles", bufs=1) as sing,
            tc.tile_pool(name="wg", bufs=2) as wgp,
            tc.tile_pool(name="wgd", bufs=2) as wgdp,
            tc.tile_pool(name="wout", bufs=2) as wwp,
            tc.tile_pool(name="tmp", bufs=1) as tmpp,
            tc.tile_pool(name="ps", bufs=1, space="PSUM") as ps,
        ):
            # ---- constants ----
            eps_t = sing.tile([1, 1], dt.float32)
            nc.vector.memset(eps_t[:], 1e-5)
            ln16_t = sing.tile([1, 1], dt.float32)
            nc.vector.memset(ln16_t[:], float(np.log(S_X)))
            ones8 = sing.tile([128, 1], dt.float8e4)
            nc.vector.memset(ones8[:], 1.0)
            ones1_bf = sing.tile([1, 128], dt.bfloat16)
            nc.vector.memset(ones1_bf[:], 1.0)
            pm_w = sing.tile([XPW, 128], dt.bfloat16)    # +-1/4 rows for s dot
            nc.vector.memset(pm_w[64:XPW, :], 0.0)
            nc.vector.memset(pm_w[64:80, :], 0.25)
            nc.vector.memset(pm_w[96:112, :], -0.25)
            junk = sing.tile([1, 8], dt.float32)
            nc.vector.memset(junk[:], 0.0)

            def act_dummy(func, dep=None):
                # tiny op to pull the ACT table load into engine slack;
                # dep (an AP) orders it after the producer so the tile
                # scheduler cannot hoist it ahead of the previous table era
                src_ap = junk[:] if dep is None else dep
                nc.scalar.activation(junk[:], src_ap, func)

            act_dummy(AF.Tanh)     # pre-load the silu/tanh table set at t=0

            # ---- resident small weights / inputs ----
            # proj weights + input share the dense-weight pool buffers
            # (dead after proj phase; dense DMAs recycle them)
            xw1_sb = wgdp.tile([128, KIN, BL + D // 2], dt.bfloat16, tag="wgd",
                               name="xw1")
            x_sb = xw1_sb[:, :, :BL]
            wp1_sb = xw1_sb[:, :, BL:]
            for k in range(KIN):
                nc.sync.dma_start(x_sb[:, k, :], x_in[:, k])
                nc.sync.dma_start(wp1_sb[:, k, :], w_p1[:, k])
            wp2_sb = wgdp.tile([128, KIN, D], dt.bfloat16, tag="wgd", name="wp2")
            nc.sync.dma_start(wp2_sb[:], w_p2)
            bp1_sb = sing.tile([128, KIN], dt.float32)
            nc.sync.dma_start(bp1_sb[:], b_p1)
            bp2_sb = sing.tile([128, KD], dt.float32)
            nc.sync.dma_start(bp2_sb[:], b_p2)
            bd1_sb = sing.tile([128, 16], dt.float32)
            nc.sync.dma_start(bd1_sb[:], b_d1)
            wd2_sb = sing.tile([128, 16, OUT], dt.bfloat16)
            nc.sync.dma_start(wd2_sb[:], w_d2)
            bd2_sb = sing.tile([128, 2], dt.float32)
            nc.sync.dma_start(bd2_sb[:], b_d2)

            # ---- persistent activations ----
            xT = sing.tile([128, KD, BL], dt.float32)       # residual stream x^T
            x_bf = sing.tile([128, KD, BL], dt.bfloat16)    # h1 scratch + dense rhs
            scr8 = sing.tile([128, KD, BL], dt.float8e4)    # squares, then xn8
            xs_bf = sing.tile([128, KDI, BL], dt.bfloat16)
            sz_bf = sing.tile([128, KDI, BL], dt.bfloat16)
            xssz = sing.tile([128, KDI, BL], dt.bfloat16)
            delta = sing.tile([128, KDI, BL], dt.bfloat16)  # also dense g1 scratch
            yin8 = sing.tile([128, KDI, BL], dt.float8e4)
            dlo_bf = sing.tile([DTR, BL], dt.bfloat16)
            sqpm = sing.tile([XPW, BL], dt.bfloat16)
            s_sb = sing.tile([128, BL], dt.bfloat16)
            rs_bf = sing.tile([1, BL], dt.bfloat16)
            lnms_t = sing.tile([1, BL], dt.float32)
            out_sb = sing.tile([128, 2, BL], dt.bfloat16)

            _psn = [0]

            def mm_ps(tag="mm", bufs=6, shape=(128, BL)):
                _psn[0] += 1
                return ps.tile(list(shape), dt.float32, tag=tag, bufs=bufs,
                               name=f"ps_{tag}_{_psn[0]}")

            # ======== proj MLP: x -> h1 -> x_T (+ squares for L0 rms) ====
            with nc.named_scope("proj_mlp"):
                for j in range(KIN):        # h1 j-tiles (D/2 = 512 -> 4)
                    pt = mm_ps()
                    for k in range(KIN):
                        nc.tensor.matmul(pt[:], wp1_sb[:, k, j * 128:(j + 1) * 128],
                                         x_sb[:, k, :],
                                         start=(k == 0), stop=(k == KIN - 1))
                    nc.scalar.activation(x_bf[:, j, :], pt[:], AF.Tanh,
                                         bias=bp1_sb[:, j:j + 1])
                h1_bf = x_bf                # h1 lives in x_bf[:, 0:4, :]
                for j in range(KD):         # h j-tiles (D = 1024 -> 8)
                    pt = mm_ps()
                    for k in range(KIN):
                        nc.tensor.matmul(pt[:], wp2_sb[:, k, j * 128:(j + 1) * 128],
                                         h1_bf[:, k, :],
                                         start=(k == 0), stop=(k == KIN - 1))
                    nc.scalar.activation(xT[:, j, :], pt[:], AF.Identity,
                                         bias=bp2_sb[:, j:j + 1])
                    nc.vector.scalar_tensor_tensor(
                        scr8[:, j, :], xT[:, j, :], 4.0, xT[:, j, :],
                        ALU.mult, ALU.mult)

            # ======== mamba layers ========
            for l in range(NL):
                with nc.named_scope(f"L{l}_pre"):
                    wxp = tmpp.tile([128, KDI, XPW], dt.bfloat16, tag="wxp")
                    nc.sync.dma_start(wxp[:], w_xp[l])
                    wdt = tmpp.tile([DTR, KDI, 128], dt.bfloat16, tag="wdt")
                    nc.sync.dma_start(wdt[:], w_dt[l])
                    bcv = tmpp.tile([128, KDI], dt.float32, tag="bcv")
                    nc.sync.dma_start(bcv[:], b_cv[l])
                    bdt = tmpp.tile([128, KDI], dt.float32, tag="bdt")
                    nc.sync.dma_start(bdt[:], b_dt[l])
                    dsm = tmpp.tile([128, KDI], dt.float32, tag="dsm")
                    nc.sync.dma_start(dsm[:], d_ssm[l])
                    wout = wwp.tile([128, KDI, 1024], dt.float8e4, tag="wout")
                    nc.sync.dma_start(wout[:], w_out[l])

                    # rms stats on fp8 squares; xn8 = xT * (rs*16) in fp8
                    pssq = mm_ps(tag="small", bufs=2, shape=(1, BL))
                    for k in range(KD):
                        nc.tensor.matmul(pssq[:], ones8[:], scr8[:, k, :],
                                         start=(k == 0), stop=(k == KD - 1))
                    nc.scalar.activation(lnms_t[:], pssq[:], AF.Ln,
                                         bias=eps_t[:], scale=1.0 / (4 * D))
                    nc.scalar.activation(rs_bf[:], lnms_t[:], AF.Exp,
                                         bias=ln16_t[:], scale=-0.5)
                    act_dummy(AF.Silu, rs_bf[:, 0:8])
                    ps_rs = mm_ps(tag="small", bufs=2)
                    nc.tensor.matmul(ps_rs[:], ones1_bf[:], rs_bf[:],
                                     start=True, stop=True)
                    xn8 = scr8
                    for k in range(KD):
                        nc.vector.tensor_mul(xn8[:, k, :], xT[:, k, :], ps_rs[:])

                # --- in_proj: fp8 DoubleRow, descale folded into silu evac ---
                def xproj_chain():
                    # x_proj (bf16) -> dlo | p | m ; s = (p^2 - m^2)/4.
                    # Identity/Square run in any ACT table era, so this
                    # overlaps the z-half silu block without table thrash.
                    with nc.named_scope(f"L{l}_xproj"):
                        pdb = mm_ps(tag="small", bufs=2, shape=(XPW, BL))
                        for k in range(KDI):
                            nc.tensor.matmul(pdb[:], wxp[:, k, :], xs_bf[:, k, :],
                                             start=(k == 0), stop=(k == KDI - 1))
                        nc.scalar.activation(dlo_bf[:], pdb[:DTR, :], AF.Identity)
                        nc.scalar.activation(sqpm[64:XPW, :], pdb[64:XPW, :],
                                             AF.Square)
                        ps_s = mm_ps(tag="small", bufs=2)
                        nc.tensor.matmul(ps_s[:], pm_w[64:XPW, :], sqpm[64:XPW, :],
                                         start=True, stop=True)
                        nc.scalar.activation(s_sb[:], ps_s[:], AF.Identity)

                with nc.named_scope(f"L{l}_inproj"):
                    for g in range(NG):
                        if g == 2:
                            xproj_chain()
                        wg = wgp.tile([128, KD, 1024], dt.float8e4, tag="wg")
                        nc.sync.dma_start(wg[:], w_in[l][g])
                        for jj in range(GJ):
                            pt = mm_ps(tag="mm", bufs=6)
                            for kk in range(0, KD, 2):
                                nc.tensor.matmul(
                                    pt[:], wg[:, kk:kk + 2, jj * 128:(jj + 1) * 128],
                                    xn8[:, kk:kk + 2, :],
                                    start=(kk == 0), stop=(kk == KD - 2),
                                    perf_mode=DR)
                            j = g * GJ + jj
                            if j < KDI:
                                nc.scalar.activation(xs_bf[:, j, :], pt[:], AF.Silu,
                                                     bias=bcv[:, j:j + 1],
                                                     scale=1.0 / (S_XS * S_X))
                            else:
                                nc.scalar.activation(sz_bf[:, j - KDI, :], pt[:],
                                                     AF.Silu,
                                                     scale=1.0 / (S_Z * S_X))

                with nc.named_scope(f"L{l}_xssz"):
                    # split between DVE (fast all-bf16 2x path) and GpSimd so
                    # neither lane gates the downstream STT chain
                    for k in range(KDI):
                        eng = nc.vector if k % 2 == 0 else nc.gpsimd
                        eng.tensor_mul(xssz[:, k, :], xs_bf[:, k, :],
                                       sz_bf[:, k, :])

                if l == 1:
                    # prefetch dense-MLP weights early; pool buffers are the
                    # recycled proj-weight buffers, free since the proj phase
                    dense_wg = []
                    for g in range(2):
                        wgd = wgdp.tile([128, KD, 1024], dt.bfloat16, tag="wgd",
                                        name=f"dense_wg{g}")
                        nc.sync.dma_start(wgd[:], w_d1[g])
                        dense_wg.append(wgd)

                # --- dt (bf16): softplus = Ln(Exp(u+b)+1), Ln paired ---
                with nc.named_scope(f"L{l}_dt"):
                    act_dummy(AF.Exp, sz_bf[0:1, KDI - 1, 0:8])
                    for j in range(KDI):
                        pt = mm_ps()
                        nc.tensor.matmul(pt[:], wdt[:, j, :], dlo_bf[:],
                                         start=True, stop=True)
                        nc.scalar.activation(delta[:, j, :], pt[:], AF.Exp,
                                             bias=bdt[:, j:j + 1])
                        if j % 2 == 1:
                            nc.scalar.activation(delta[:, j - 1:j + 1, :],
                                                 delta[:, j - 1:j + 1, :],
                                                 AF.Ln, bias=1.0)
                            for jj in (j - 1, j):
                                nc.vector.tensor_mul(delta[:, jj, :],
                                                     delta[:, jj, :], s_sb[:])
                            for jj in (j - 1, j):
                                nc.vector.scalar_tensor_tensor(
                                    delta[:, jj, :], delta[:, jj, :],
                                    dsm[:, jj:jj + 1], xssz[:, jj, :],
                                    ALU.add, ALU.mult)
                            nc.vector.tensor_copy(yin8[:, j - 1:j + 1, :],
                                                  delta[:, j - 1:j + 1, :])

                # --- y @ out_w (fp8 DR) + residual; squares for next rms ---
                with nc.named_scope(f"L{l}_y_out"):
                    NP1 = 6
                    pouts = [mm_ps() for _ in range(NP1)]
                    for kk in range(0, KDI, 2):
                        for j in range(NP1):
                            nc.tensor.matmul(pouts[j][:],
                                             wout[:, kk:kk + 2, j * 128:(j + 1) * 128],
                                             yin8[:, kk:kk + 2, :],
                                             start=(kk == 0), stop=(kk == KDI - 2),
                                             perf_mode=DR)

                    def evac_y(j, pt):
                        if l < NL - 1:
                            nc.vector.scalar_tensor_tensor(
                                xT[:, j, :], pt[:], 1.0 / S_OUT, xT[:, j, :],
                                ALU.mult, ALU.add)
                            nc.scalar.activation(scr8[:, j, :], xT[:, j, :],
                                                 AF.Square, scale=2.0)
                        else:
                            nc.vector.scalar_tensor_tensor(
                                x_bf[:, j, :], pt[:], 1.0 / S_OUT, xT[:, j, :],
                                ALU.mult, ALU.add)

                    for j in range(NP1):
                        evac_y(j, pouts[j])
                    for j in range(NP1, KD):
                        pt = mm_ps()
                        for kk in range(0, KDI, 2):
                            nc.tensor.matmul(pt[:],
                                             wout[:, kk:kk + 2, j * 128:(j + 1) * 128],
                                             yin8[:, kk:kk + 2, :],
                                             start=(kk == 0), stop=(kk == KDI - 2),
                                             perf_mode=DR)
                        evac_y(j, pt)

            # ======== dense MLP (bf16): x -> g1 -> out ========
            with nc.named_scope("dense_mlp"):
                act_dummy(AF.Tanh, x_bf[0:1, KD - 1, 0:8])
                g1_bf = delta               # reuse [128, KDI, BL] bf16 scratch
                for g in range(2):
                    wgd = dense_wg[g]
                    for jj in range(GJ):
                        pt = mm_ps()
                        for k in range(KD):
                            nc.tensor.matmul(pt[:], wgd[:, k, jj * 128:(jj + 1) * 128],
                                             x_bf[:, k, :],
                                             start=(k == 0), stop=(k == KD - 1))
                        j = g * GJ + jj
                        nc.scalar.activation(g1_bf[:, j, :], pt[:], AF.Tanh,
                                             bias=bd1_sb[:, j:j + 1])
                for j in range(2):
                    pt = mm_ps()
                    for k in range(16):
                        nc.tensor.matmul(pt[:], wd2_sb[:, k, j * 128:(j + 1) * 128],
                                         g1_bf[:, k, :], start=(k == 0),
                                         stop=(k == 15))
                    nc.scalar.activation(out_sb[:, j, :], pt[:], AF.Tanh,
                                         bias=bd2_sb[:, j:j + 1])
                    nc.gpsimd.dma_start(out_d[j], out_sb[:, j, :])

    nc.compile()
    return nc


def _run(inputs, trace=False, trace_kwargs=None):
    if "nc" not in _cache:
        _cache["nc"] = _build()
    nc = _cache["nc"]
    p, xc = _host_pack(inputs)
    in_maps = []
    for c in range(NCORES):
        m = dict(p)
        m["x_in"] = xc[c]
        in_maps.append(m)

    from concourse.bass_utils import run_bass_kernel_spmd
    kw = {}
    if trace:
        kw.update(trace=True, trace_cores=[0], trace_kwargs=trace_kwargs or {})
    res = run_bass_kernel_spmd(nc, in_maps, core_ids=list(range(NCORES)), **kw)

    # assemble: per core out [2, 128, BL] -> out^T [256, BL] -> [BL, 256]
    full = np.empty((B, OUT), np.float32)
    for c in range(NCORES):
        o = res.results[c]["out"].reshape(OUT, BL).astype(np.float32)
        full[c * BL:(c + 1) * BL] = o.T
    return full.reshape(-1), res


def kernel(**inputs):
    out, _ = _run(inputs, trace=False)
    return out


# revision 19
# speedup vs baseline: 1.0734x; 1.0121x over previous
# Trainium2 Bass kernel for nn_Net_38233798869763 (Mamba-ish net, L=1).
#
# Math (L=1 collapses the reference):
#   - causal depthwise conv over L=1 reduces to xc = xs0*conv_w[:,3] + conv_b
#   - the SSM scan reduces to y_ssm = delta * xs * (Bm . Cm)   (dA hits h0=0)
#   so each layer is:
#     rs   = rsqrt(mean(x^2) + eps)
#     xn   = x * rs                                  (norm_w folded into W_in)
#     xs   = silu(xn @ W_xs.T + conv_b); sz = silu(xn @ W_z.T)
#     dbl  = xs @ x_proj_w.T; dlo, Bm, Cm = split(dbl)
#     s    = sum(Bm*Cm) = ((Bm+Cm)^2 - (Bm-Cm)^2)/4  (x_proj folded to p/m cols
#                                                     so the dot is all PE ops)
#     delta= softplus(dlo @ dt_w.T + dt_b)           (= Ln(Exp(u)+1) on ACT)
#     x   += ((delta*s + D_ssm) * xs * sz) @ out_w.T
#
# Precision plan (validated by host sim, rel_l2 ~1.1e-2 < 2e-2):
#   - trunk matmuls (in_proj / x_proj / out_proj) in fp8e4 with DoubleRow
#     (2 fp8 weights per PE cell -> ~1.4x matmul throughput), weights
#     host-scaled by power-of-2 per matrix, descale folded into the psum
#     evacuation activations; activations quantized to fp8 at evac time.
#   - base-signal matmuls (proj MLP, dt, dense MLP) stay bf16: fp8 there
#     costs ~4% output error (no residual damping), bf16 is cheap (44us).
# Batch sharded across 8 cores (512 rows/core), feature-on-partitions.
import numpy as np
import ml_dtypes

B, IN, D, OUT = 4096, 512, 1024, 256
NL, DI, N, DCONV, DTR = 4, 2048, 16, 4, 64
NCORES = 8
BL = B // NCORES          # 512 batch rows per core
KD = D // 128             # 8   k-tiles over D
KIN = IN // 128           # 4   k-tiles over IN
KDI = DI // 128           # 16  k-tiles over DI
JI = 2 * DI // 128        # 32  j-tiles of in_proj output
GJ = 8                    # j-tiles per psum group
NG = JI // GJ             # 4   groups (2 xs + 2 z)

XPW = DTR + 3 * N        # x_proj out cols: dlo|p|pad|m (32-aligned starts)
S_XS = 2048.0             # fp8 scale: in_proj xs half (conv tap folded in)
S_Z = 256.0               # fp8 scale: in_proj z half
S_XP = 256.0              # fp8 scale: x_proj
S_OUT = 2048.0            # fp8 scale: out_proj
S_X = 16.0                # fp8 scale of normalized activations (via rs)

_cache = {}


def _q8(a, s):
    f8 = ml_dtypes.float8_e4m3
    return np.clip(np.asarray(a, np.float32) * s, -240.0, 240.0).astype(f8)


def _host_pack(inputs):
    bfl = ml_dtypes.bfloat16
    f32 = np.float32

    def t(a):
        return np.ascontiguousarray(a)

    p = {}
    # proj MLP (bf16)
    p["w_p1"] = t(inputs["pw1"].T.reshape(KIN, 128, D // 2).transpose(1, 0, 2).astype(bfl))
    p["b_p1"] = t(inputs["pb1"].reshape(D // 2 // 128, 128).T.astype(f32))
    p["w_p2"] = t(inputs["pw2"].T.reshape(KIN, 128, D).transpose(1, 0, 2).astype(bfl))
    p["b_p2"] = t(inputs["pb2"].reshape(KD, 128).T.astype(f32))
    # dense MLP (bf16)
    dw1T = inputs["dw1"].T            # [D, 2D]
    p["w_d1"] = t(np.stack([
        dw1T[:, g * 1024:(g + 1) * 1024].reshape(KD, 128, 1024).transpose(1, 0, 2)
        for g in range(2)
    ]).astype(bfl))                   # [2, 128, 8, 1024]
    p["b_d1"] = t(inputs["db1"].reshape(16, 128).T.astype(f32))
    p["w_d2"] = t(inputs["dw2"].T.reshape(16, 128, OUT).transpose(1, 0, 2).astype(bfl))
    p["b_d2"] = t(inputs["db2"].reshape(2, 128).T.astype(f32))
    # per-layer mamba params
    for l in range(NL):
        W_in = inputs["in_proj_w"][l] * inputs["norm_w"][l][None, :]
        W_in = W_in.copy()
        W_in[:DI] *= inputs["conv_w"][l][:, DCONV - 1][:, None]   # fold last conv tap
        W_in[:DI] *= S_XS
        W_in[DI:] *= S_Z
        WT = np.clip(W_in, -240.0, 240.0).T                       # [D, 2*DI] scaled
        p[f"w_in{l}"] = t(np.stack([
            WT[:, g * 1024:(g + 1) * 1024].reshape(KD, 128, 1024).transpose(1, 0, 2)
            for g in range(NG)
        ]).astype(ml_dtypes.float8_e4m3))                         # [4, 128, 8, 1024] fp8
        # x_proj folded: [dlo(64) | Bm+Cm(16) | Bm-Cm(16)]
        Wxp = inputs["x_proj_w"][l]
        Wxp_pm = np.concatenate([Wxp[:DTR],
                                 Wxp[DTR:DTR + N] + Wxp[DTR + N:],
                                 np.zeros((N, DI), np.float32),
                                 Wxp[DTR:DTR + N] - Wxp[DTR + N:]], axis=0)
        p[f"w_xp{l}"] = t(Wxp_pm.T.astype(bfl).reshape(KDI, 128, XPW)
                          .transpose(1, 0, 2))                    # [128, 16, 112] bf16
        p[f"w_dt{l}"] = t(inputs["dt_w"][l].T.reshape(DTR, KDI, 128).astype(bfl))
        p[f"w_out{l}"] = t(_q8(inputs["out_w"][l].T, S_OUT).reshape(KDI, 128, D)
                           .transpose(1, 0, 2))                   # [128, 16, 1024] fp8
        p[f"b_cv{l}"] = t(inputs["conv_b"][l].reshape(KDI, 128).T.astype(f32))
        p[f"b_dt{l}"] = t(inputs["dt_b"][l].reshape(KDI, 128).T.astype(f32))
        p[f"d_ssm{l}"] = t(inputs["D_ssm"][l].reshape(KDI, 128).T.astype(f32))
    # input, transposed + per-core sliced: x^T [IN, B] -> [core][128, KIN, BL]
    xT = inputs["x"].T.astype(bfl)
    xc = []
    for c in range(NCORES):
        s = xT[:, c * BL:(c + 1) * BL].reshape(KIN, 128, BL).transpose(1, 0, 2)
        xc.append(t(s))
    return p, xc


def _patch_act_tables():
    """Steer the ACT table-set chooser: Exp+Ln co-reside (softplus + rms
    stats both live in natural_log_exp_and_others), Tanh+Silu co-reside in
    silu_and_others. Dict ORDER and SIZE must stay identical to
    act_info.json (set ids are positional); only MEMBERSHIP is edited."""
    import concourse.mybir as mybir
    import concourse.bacc as bacc_mod
    if getattr(bacc_mod, "_act_tables_patched", False):
        return
    orig = bacc_mod.get_activation_tables
    AF = mybir.ActivationFunctionType

    def steered(module_arch):
        tabs = orig(module_arch)
        keep = "natural_log_exp_and_others"
        for name, fns in tabs.items():
            if name != keep:
                fns.discard(AF.Exp)
                fns.discard(AF.Ln)
            if name != "silu_and_others":
                fns.discard(AF.Tanh)
        return tabs

    bacc_mod.get_activation_tables = steered
    bacc_mod._act_tables_patched = True


def _build():
    import concourse.tile as tile
    import concourse.mybir as mybir
    from concourse import bacc

    _patch_act_tables()

    dt = mybir.dt
    AF = mybir.ActivationFunctionType
    ALU = mybir.AluOpType
    DR = mybir.MatmulPerfMode.DoubleRow

    nc = bacc.Bacc("TRN2", target_bir_lowering=False, debug=False,
                   num_devices=NCORES)

    def din(name, shape, dtp):
        return nc.dram_tensor(name, shape, dtp, kind="ExternalInput").ap()

    x_in = din("x_in", [128, KIN, BL], dt.bfloat16)
    w_p1 = din("w_p1", [128, KIN, D // 2], dt.bfloat16)
    b_p1 = din("b_p1", [128, KIN], dt.float32)
    w_p2 = din("w_p2", [128, KIN, D], dt.bfloat16)
    b_p2 = din("b_p2", [128, KD], dt.float32)
    w_d1 = din("w_d1", [2, 128, KD, 1024], dt.bfloat16)
    b_d1 = din("b_d1", [128, 16], dt.float32)
    w_d2 = din("w_d2", [128, 16, OUT], dt.bfloat16)
    b_d2 = din("b_d2", [128, 2], dt.float32)
    w_in = [din(f"w_in{l}", [NG, 128, KD, 1024], dt.float8e4) for l in range(NL)]
    w_xp = [din(f"w_xp{l}", [128, KDI, XPW], dt.bfloat16) for l in range(NL)]
    w_dt = [din(f"w_dt{l}", [DTR, KDI, 128], dt.bfloat16) for l in range(NL)]
    w_out = [din(f"w_out{l}", [128, KDI, 1024], dt.float8e4) for l in range(NL)]
    b_cv = [din(f"b_cv{l}", [128, KDI], dt.float32) for l in range(NL)]
    b_dt = [din(f"b_dt{l}", [128, KDI], dt.float32) for l in range(NL)]
    d_ssm = [din(f"d_ssm{l}", [128, KDI], dt.float32) for l in range(NL)]
    out_d = nc.dram_tensor("out", [2, 128, BL], dt.bfloat16, kind="ExternalOutput").ap()

    with tile.TileContext(nc) as tc:
        with (
            tc.tile_pool(name="singles", bufs=1) as sing,
            tc.tile_pool(name="wg", bufs=2) as wgp,
            tc.tile_pool(name="wgd", bufs=2) as wgdp,
            tc.tile_pool(name="wout", bufs=2) as wwp,
            tc.tile_pool(name="tmp", bufs=1) as tmpp,
            tc.tile_pool(name="ps", bufs=1, space="PSUM") as ps,
        ):
            # ---- constants ----
            eps_t = sing.tile([1, 1], dt.float32)
            nc.vector.memset(eps_t[:], 1e-5)
            ln16_t = sing.tile([1, 1], dt.float32)
            nc.vector.memset(ln16_t[:], float(np.log(S_X)))
            ones8 = sing.tile([128, 1], dt.float8e4)
            nc.vector.memset(ones8[:], 1.0)
            ones1_bf = sing.tile([1, 128], dt.bfloat16)
            nc.vector.memset(ones1_bf[:], 1.0)
            pm_w = sing.tile([XPW, 128], dt.bfloat16)    # +-1/4 rows for s dot
            nc.vector.memset(pm_w[64:XPW, :], 0.0)
            nc.vector.memset(pm_w[64:80, :], 0.25)
            nc.vector.memset(pm_w[96:112, :], -0.25)
            junk = sing.tile([1, 8], dt.float32)
            nc.vector.memset(junk[:], 0.0)

            def act_dummy(func, dep=None):
                # tiny op to pull the ACT table load into engine slack;
                # dep (an AP) orders it after the producer so the tile
                # scheduler cannot hoist it ahead of the previous table era
                src_ap = junk[:] if dep is None else dep
                nc.scalar.activation(junk[:], src_ap, func)

            act_dummy(AF.Tanh)     # pre-load the silu/tanh table set at t=0

            # ---- resident small weights / inputs ----
            # small bias DMAs first so they don't queue behind the megabyte
            # weight transfers (the first tanh evac needs b_p1)
            bp1_sb = sing.tile([128, KIN], dt.float32)
            nc.sync.dma_start(bp1_sb[:], b_p1)
            bp2_sb = sing.tile([128, KD], dt.float32)
            nc.sync.dma_start(bp2_sb[:], b_p2)
            bd1_sb = sing.tile([128, 16], dt.float32)
            nc.sync.dma_start(bd1_sb[:], b_d1)
            bd2_sb = sing.tile([128, 2], dt.float32)
            nc.sync.dma_start(bd2_sb[:], b_d2)
            # proj weights + input share the dense-weight pool buffers
            # (dead after proj phase; dense DMAs recycle them)
            xw1_sb = wgdp.tile([128, KIN, BL + D // 2], dt.bfloat16, tag="wgd",
                               name="xw1")
            x_sb = xw1_sb[:, :, :BL]
            wp1_sb = xw1_sb[:, :, BL:]
            for k in range(KIN):
                nc.sync.dma_start(x_sb[:, k, :], x_in[:, k])
                nc.sync.dma_start(wp1_sb[:, k, :], w_p1[:, k])
            wp2_sb = wgdp.tile([128, KIN, D], dt.bfloat16, tag="wgd", name="wp2")
            nc.sync.dma_start(wp2_sb[:], w_p2)
            wd2_sb = sing.tile([128, 16, OUT], dt.bfloat16)
            nc.sync.dma_start(wd2_sb[:], w_d2)

            # ---- persistent activations ----
            xT = sing.tile([128, KD, BL], dt.float32)       # residual stream x^T
            x_bf = sing.tile([128, KD, BL], dt.bfloat16)    # h1 scratch + dense rhs
            scr8 = sing.tile([128, KD, BL], dt.float8e4)    # squares, then xn8
            xs_bf = sing.tile([128, KDI, BL], dt.bfloat16)
            sz_bf = sing.tile([128, KDI, BL], dt.bfloat16)
            xssz = sing.tile([128, KDI, BL], dt.bfloat16)
            delta = sing.tile([128, KDI, BL], dt.bfloat16)  # also dense g1 scratch
            yin8 = sing.tile([128, KDI, BL], dt.float8e4)
            dlo_bf = sing.tile([DTR, BL], dt.bfloat16)
            sqpm = sing.tile([XPW, BL], dt.bfloat16)
            s_sb = sing.tile([128, BL], dt.bfloat16)
            rs_bf = sing.tile([1, BL], dt.bfloat16)
            lnms_t = sing.tile([1, BL], dt.float32)
            out_sb = sing.tile([128, 2, BL], dt.bfloat16)

            _psn = [0]

            def mm_ps(tag="mm", bufs=6, shape=(128, BL)):
                _psn[0] += 1
                return ps.tile(list(shape), dt.float32, tag=tag, bufs=bufs,
                               name=f"ps_{tag}_{_psn[0]}")

            # ======== proj MLP: x -> h1 -> x_T (+ squares for L0 rms) ====
            with nc.named_scope("proj_mlp"):
                for j in range(KIN):        # h1 j-tiles (D/2 = 512 -> 4)
                    pt = mm_ps()
                    for k in range(KIN):
                        nc.tensor.matmul(pt[:], wp1_sb[:, k, j * 128:(j + 1) * 128],
                                         x_sb[:, k, :],
                                         start=(k == 0), stop=(k == KIN - 1))
                    nc.scalar.activation(x_bf[:, j, :], pt[:], AF.Tanh,
                                         bias=bp1_sb[:, j:j + 1])
                h1_bf = x_bf                # h1 lives in x_bf[:, 0:4, :]
                for j in range(KD):         # h j-tiles (D = 1024 -> 8)
                    pt = mm_ps()
                    for k in range(KIN):
                        nc.tensor.matmul(pt[:], wp2_sb[:, k, j * 128:(j + 1) * 128],
                                         h1_bf[:, k, :],
                                         start=(k == 0), stop=(k == KIN - 1))
                    nc.scalar.activation(xT[:, j, :], pt[:], AF.Identity,
                                         bias=bp2_sb[:, j:j + 1])
                    nc.vector.scalar_tensor_tensor(
                        scr8[:, j, :], xT[:, j, :], 4.0, xT[:, j, :],
                        ALU.mult, ALU.mult)

            # ======== mamba layers ========
            for l in range(NL):
                with nc.named_scope(f"L{l}_pre"):
                    wxp = tmpp.tile([128, KDI, XPW], dt.bfloat16, tag="wxp")
                    nc.sync.dma_start(wxp[:], w_xp[l])
                    wdt = tmpp.tile([DTR, KDI, 128], dt.bfloat16, tag="wdt")
                    nc.sync.dma_start(wdt[:], w_dt[l])
                    bcv = tmpp.tile([128, KDI], dt.float32, tag="bcv")
                    nc.sync.dma_start(bcv[:], b_cv[l])
                    bdt = tmpp.tile([128, KDI], dt.float32, tag="bdt")
                    nc.sync.dma_start(bdt[:], b_dt[l])
                    dsm = tmpp.tile([128, KDI], dt.float32, tag="dsm")
                    nc.sync.dma_start(dsm[:], d_ssm[l])
                    wout = wwp.tile([128, KDI, 1024], dt.float8e4, tag="wout")
                    nc.sync.dma_start(wout[:], w_out[l])

                    # rms stats on fp8 squares; xn8 = xT * (rs*16) in fp8
                    pssq = mm_ps(tag="small", bufs=2, shape=(1, BL))
                    for k in range(KD):
                        nc.tensor.matmul(pssq[:], ones8[:], scr8[:, k, :],
                                         start=(k == 0), stop=(k == KD - 1))
                    nc.scalar.activation(lnms_t[:], pssq[:], AF.Ln,
                                         bias=eps_t[:], scale=1.0 / (4 * D))
                    nc.scalar.activation(rs_bf[:], lnms_t[:], AF.Exp,
                                         bias=ln16_t[:], scale=-0.5)
                    act_dummy(AF.Silu, rs_bf[:, 0:8])
                    ps_rs = mm_ps(tag="small", bufs=2)
                    nc.tensor.matmul(ps_rs[:], ones1_bf[:], rs_bf[:],
                                     start=True, stop=True)
                    xn8 = scr8
                    for k in range(KD):
                        nc.vector.tensor_mul(xn8[:, k, :], xT[:, k, :], ps_rs[:])

                # --- in_proj: fp8 DoubleRow, descale folded into silu evac ---
                def xproj_chain():
                    # x_proj (bf16) -> dlo | p | m ; s = (p^2 - m^2)/4.
                    # Identity/Square run in any ACT table era, so this
                    # overlaps the z-half silu block without table thrash.
                    with nc.named_scope(f"L{l}_xproj"):
                        pdb = mm_ps(tag="small", bufs=2, shape=(XPW, BL))
                        for k in range(KDI):
                            nc.tensor.matmul(pdb[:], wxp[:, k, :], xs_bf[:, k, :],
                                             start=(k == 0), stop=(k == KDI - 1))
                        nc.scalar.activation(dlo_bf[:], pdb[:DTR, :], AF.Identity)
                        nc.scalar.activation(sqpm[64:XPW, :], pdb[64:XPW, :],
                                             AF.Square)
                        ps_s = mm_ps(tag="small", bufs=2)
                        nc.tensor.matmul(ps_s[:], pm_w[64:XPW, :], sqpm[64:XPW, :],
                                         start=True, stop=True)
                        nc.scalar.activation(s_sb[:], ps_s[:], AF.Identity)

                with nc.named_scope(f"L{l}_inproj"):
                    for g in range(NG):
                        if g == 2:
                            xproj_chain()
                        wg = wgp.tile([128, KD, 1024], dt.float8e4, tag="wg")
                        nc.sync.dma_start(wg[:], w_in[l][g])
                        for jj in range(GJ):
                            pt = mm_ps(tag="mm", bufs=6)
                            for kk in range(0, KD, 2):
                                nc.tensor.matmul(
                                    pt[:], wg[:, kk:kk + 2, jj * 128:(jj + 1) * 128],
                                    xn8[:, kk:kk + 2, :],
                                    start=(kk == 0), stop=(kk == KD - 2),
                                    perf_mode=DR)
                            j = g * GJ + jj
                            if j < KDI:
                                nc.scalar.activation(xs_bf[:, j, :], pt[:], AF.Silu,
                                                     bias=bcv[:, j:j + 1],
                                                     scale=1.0 / (S_XS * S_X))
                            else:
                                nc.scalar.activation(sz_bf[:, j - KDI, :], pt[:],
                                                     AF.Silu,
                                                     scale=1.0 / (S_Z * S_X))

                with nc.named_scope(f"L{l}_xssz"):
                    # split between DVE (fast all-bf16 2x path) and GpSimd so
                    # neither lane gates the downstream STT chain
                    for k in range(KDI):
                        eng = nc.vector if k % 2 == 0 else nc.gpsimd
                        eng.tensor_mul(xssz[:, k, :], xs_bf[:, k, :],
                                       sz_bf[:, k, :])

                if l == 1:
                    # prefetch dense-MLP weights early; pool buffers are the
                    # recycled proj-weight buffers, free since the proj phase
                    dense_wg = []
                    for g in range(2):
                        wgd = wgdp.tile([128, KD, 1024], dt.bfloat16, tag="wgd",
                                        name=f"dense_wg{g}")
                        nc.sync.dma_start(wgd[:], w_d1[g])
                        dense_wg.append(wgd)

                # --- dt (bf16): softplus = Ln(Exp(u+b)+1), Ln paired ---
                with nc.named_scope(f"L{l}_dt"):
                    act_dummy(AF.Exp, sz_bf[0:1, KDI - 1, 0:8])
                    for j in range(KDI):
                        pt = mm_ps()
                        nc.tensor.matmul(pt[:], wdt[:, j, :], dlo_bf[:],
                                         start=True, stop=True)
                        nc.scalar.activation(delta[:, j, :], pt[:], AF.Exp,
                                             bias=bdt[:, j:j + 1])
                        if j % 2 == 1:
                            nc.scalar.activation(delta[:, j - 1:j + 1, :],
                                                 delta[:, j - 1:j + 1, :],
                                                 AF.Ln, bias=1.0)
                            for jj in (j - 1, j):
                                nc.vector.tensor_mul(delta[:, jj, :],
                                                     delta[:, jj, :], s_sb[:])
                            for jj in (j - 1, j):
                                nc.vector.scalar_tensor_tensor(
                                    delta[:, jj, :], delta[:, jj, :],
                                    dsm[:, jj:jj + 1], xssz[:, jj, :],
                                    ALU.add, ALU.mult)
                            nc.vector.tensor_copy(yin8[:, j - 1:j + 1, :],
                                                  delta[:, j - 1:j + 1, :])

                # --- y @ out_w (fp8 DR) + residual; squares for next rms ---
                with nc.named_scope(f"L{l}_y_out"):
                    NP1 = 6
                    pouts = [mm_ps() for _ in range(NP1)]
                    for kk in range(0, KDI, 2):
                        for j in range(NP1):
                            nc.tensor.matmul(pouts[j][:],
                                             wout[:, kk:kk + 2, j * 128:(j + 1) * 128],
                                             yin8[:, kk:kk + 2, :],
                                             start=(kk == 0), stop=(kk == KDI - 2),
                                             perf_mode=DR)

                    def evac_y(j, pt):
                        if l < NL - 1:
                            nc.vector.scalar_tensor_tensor(
                                xT[:, j, :], pt[:], 1.0 / S_OUT, xT[:, j, :],
                                ALU.mult, ALU.add)
                            nc.scalar.activation(scr8[:, j, :], xT[:, j, :],
                                                 AF.Square, scale=2.0)
                        else:
                            nc.vector.scalar_tensor_tensor(
                                x_bf[:, j, :], pt[:], 1.0 / S_OUT, xT[:, j, :],
                                ALU.mult, ALU.add)

                    for j in range(NP1):
                        evac_y(j, pouts[j])
                    for j in range(NP1, KD):
                        pt = mm_ps()
                        for kk in range(0, KDI, 2):
                            nc.tensor.matmul(pt[:],
                                             wout[:, kk:kk + 2, j * 128:(j + 1) * 128],
                                             yin8[:, kk:kk + 2, :],
                                             start=(kk == 0), stop=(kk == KDI - 2),
                                             perf_mode=DR)
                        evac_y(j, pt)

            # ======== dense MLP (bf16): x -> g1 -> out ========
            with nc.named_scope("dense_mlp"):
                act_dummy(AF.Tanh, x_bf[0:1, KD - 1, 0:8])
                g1_bf = delta               # reuse [128, KDI, BL] bf16 scratch
                for g in range(2):
                    wgd = dense_wg[g]
                    for jj in range(GJ):
                        pt = mm_ps()
                        for k in range(KD):
                            nc.tensor.matmul(pt[:], wgd[:, k, jj * 128:(jj + 1) * 128],
                                             x_bf[:, k, :],
                                             start=(k == 0), stop=(k == KD - 1))
                        j = g * GJ + jj
                        nc.scalar.activation(g1_bf[:, j, :], pt[:], AF.Tanh,
                                             bias=bd1_sb[:, j:j + 1])
                for j in range(2):
                    pt = mm_ps()
                    for k in range(16):
                        nc.tensor.matmul(pt[:], wd2_sb[:, k, j * 128:(j + 1) * 128],
                                         g1_bf[:, k, :], start=(k == 0),
                                         stop=(k == 15))
                    nc.scalar.activation(out_sb[:, j, :], pt[:], AF.Tanh,
                                         bias=bd2_sb[:, j:j + 1])
                    nc.gpsimd.dma_start(out_d[j], out_sb[:, j, :])

    nc.compile()
    return nc


def _run(inputs, trace=False, trace_kwargs=None):
    if "nc" not in _cache:
        _cache["nc"] = _build()
    nc = _cache["nc"]
    p, xc = _host_pack(inputs)
    in_maps = []
    for c in range(NCORES):
        m = dict(p)
        m["x_in"] = xc[c]
        in_maps.append(m)

    from concourse.bass_utils import run_bass_kernel_spmd
    kw = {}
    if trace:
        kw.update(trace=True, trace_cores=[0], trace_kwargs=trace_kwargs or {})
    res = run_bass_kernel_spmd(nc, in_maps, core_ids=list(range(NCORES)), **kw)

    # assemble: per core out [2, 128, BL] -> out^T [256, BL] -> [BL, 256]
    full = np.empty((B, OUT), np.float32)
    for c in range(NCORES):
        o = res.results[c]["out"].reshape(OUT, BL).astype(np.float32)
        full[c * BL:(c + 1) * BL] = o.T
    return full.reshape(-1), res


def kernel(**inputs):
    out, _ = _run(inputs, trace=False)
    return out
